# revision 33
# baseline (speedup 1.0000x reference)
"""Distributed Trainium2 Bass kernel: masked (upper-triangular) attention.

reference (L=4096, D=1024, fp32):
    Q = x @ Wq + bq ; K = z @ Wk + bk ; V = z @ Wv + bv
    S = Q @ K.T ; S[row > col] = -inf
    out = softmax(S / sqrt(D)) @ V

Strategy (8 NeuronCores, one TRN2 chip, SPMD):
  - Sequence parallel on query rows: core c owns rows [512c, 512c+512).
  - K/V projection sharded over z rows (512/core), AllGathered in bf16
    (K stored transposed [D, L] blocked by shard, V natural [L, D]).
  - Attention computed as S^T tiles (keys on partitions) so the P^T needed by
    the PV matmul comes straight out of the softmax with no transposes.
  - Softmax without max-subtraction (scores here are O(1), exp can't overflow
    in fp32); mask applied multiplicatively after exp, built at runtime from
    an iota constant + a per-core row0 scalar input, keeping one graph valid
    for all cores (SPMD - no per-core control flow).
  - Matmuls in bf16 with fp32 PSUM accumulation (end-to-end rel err ~3e-3).
"""

import math

import numpy as np

import concourse.mybir as mybir
import concourse.tile as tile
from concourse import bacc
from concourse.bass_utils import run_bass_kernel_spmd

F32 = mybir.dt.float32
BF16 = mybir.dt.bfloat16
AF = mybir.ActivationFunctionType
OP = mybir.AluOpType
P = 128
NCORES = 8

L = 4096
D = 1024


def build_graph(Ldim=L, Ddim=D):
    nc = bacc.Bacc("TRN2", target_bir_lowering=False, debug=False, num_devices=NCORES)
    ROWS = Ldim // NCORES        # query rows per core
    MB = ROWS // P               # 128-row m-chunks per core (4)
    ZB = ROWS // P               # z-shard 128-row blocks (4)
    SW = ROWS                    # key-tile width == z-shard width (512)
    JT = SW // P                 # 128-row subtiles per key tile (4)
    NT = NCORES                  # one key tile per shard
    IO = Ddim // P               # contraction chunks (8)
    AO = Ddim // P               # d_attn 128-blocks (8)
    VH = Ddim // 512             # 512-wide value column halves (2)
    HLF = ROWS // 256            # 256-row halves for PV psum pressure (2)
    scale = 1.0 / math.sqrt(Ddim)

    x_ext = nc.declare_dram_parameter("x", [ROWS, Ddim], F32, isOutput=False)
    z_ext = nc.declare_dram_parameter("z", [ROWS, Ddim], F32, isOutput=False)
    wq_ext = nc.declare_dram_parameter("Wq", [Ddim, Ddim], F32, isOutput=False)
    wk_ext = nc.declare_dram_parameter("Wk", [Ddim, Ddim], F32, isOutput=False)
    wv_ext = nc.declare_dram_parameter("Wv", [Ddim, Ddim], F32, isOutput=False)
    bq_ext = nc.declare_dram_parameter("bq", [Ddim], F32, isOutput=False)
    bk_ext = nc.declare_dram_parameter("bk", [Ddim], F32, isOutput=False)
    bv_ext = nc.declare_dram_parameter("bv", [Ddim], F32, isOutput=False)
    row0_ext = nc.declare_dram_parameter("row0", [1], F32, isOutput=False)
    out_ext = nc.declare_dram_parameter("out", [ROWS, Ddim], F32, isOutput=True)

    ident_d = nc.inline_tensor(np.eye(P, dtype=np.float32), name="ident_c")
    ones_d = nc.inline_tensor(np.ones((P, 8), np.float32), name="ones_c")
    # mask keeps where (m - p) + (row0 - SW*t - 128j) <= 0
    njt_np = np.broadcast_to(
        -(float(SW) * np.arange(NT)[:, None] + 128.0 * np.arange(JT)[None, :])
        .astype(np.float32).reshape(1, NT * JT), (P, NT * JT)).copy()
    njt_d = nc.inline_tensor(njt_np, name="njt_c")
    nSWt_d = nc.inline_tensor(
        np.broadcast_to((-float(SW) * np.arange(NT, dtype=np.float32))[None, :], (P, NT)).copy(),
        name="nswt_c")

    with tile.TileContext(nc) as tc:
        with tc.tile_pool(name="const", bufs=1) as constp, \
             tc.tile_pool(name="persist", bufs=1) as persist, \
             tc.tile_pool(name="dram", bufs=1, space="DRAM") as dram:
            ident = constp.tile([P, P], F32)
            nc.sync.dma_start(out=ident[:], in_=ident_d.ap())
            ones_f = constp.tile([P, 8], F32)
            nc.sync.dma_start(out=ones_f[:], in_=ones_d.ap())
            ones8 = constp.tile([P, 8], BF16)
            nc.vector.tensor_copy(ones8[:], ones_f[:])
            bvb = constp.tile([P, Ddim], F32)
            nc.sync.dma_start(out=bvb[:], in_=bv_ext[:].partition_broadcast(P))
            bqs = constp.tile([P, AO], F32)
            nc.sync.dma_start(out=bqs[:], in_=bq_ext[:].rearrange("(ao p) -> p ao", p=P))
            bks = constp.tile([P, AO], F32)
            nc.sync.dma_start(out=bks[:], in_=bk_ext[:].rearrange("(ao p) -> p ao", p=P))
            row0b = constp.tile([P, 1], F32)
            nc.sync.dma_start(out=row0b[:], in_=row0_ext[:].partition_broadcast(P))
            nswt = constp.tile([P, NT], F32)
            nc.sync.dma_start(out=nswt[:], in_=nSWt_d.ap())
            r0t = constp.tile([P, NT], F32)
            nc.vector.tensor_scalar(r0t[:], nswt[:], row0b[:], None, OP.add)

            QT = persist.tile([P, IO, ROWS], BF16)
            KW = AO * ROWS               # flat K width per partition
            VW = ZB * Ddim               # flat V width per partition
            kt_bns = [dram.tile([P, KW // 2], BF16, name=f"kt_bn{i}") for i in range(2)]
            v_bds = [dram.tile([P, VW // VH], BF16, name=f"v_bd{vh}") for vh in range(VH)]
            kt_gns = [dram.tile([NCORES, P, KW // 2], BF16, name=f"kt_gn{i}") for i in range(2)]
            v_gds = [dram.tile([NCORES, P, VW // VH], BF16, name=f"v_gd{vh}") for vh in range(VH)]

            # ------- Phase 1+2: projections of own shards; K/V AllGathered -------
            with tc.tile_pool(name="inp", bufs=1) as inp, \
                 tc.tile_pool(name="wst", bufs=3) as wst, \
                 tc.tile_pool(name="wkv", bufs=1) as wp, \
                 tc.tile_pool(name="zp", bufs=1) as zp, \
                 tc.tile_pool(name="tpp", bufs=2, space="PSUM") as tpp, \
                 tc.tile_pool(name="pp", bufs=2, space="PSUM") as pp:
                wmup = wst.tile([P, 512], BF16, tag="wm", name="wmup")
                nc.vector.memset(wmup[:], 0.0)
                wpsum = tpp.tile([P, 512], F32, tag="wm", name="wpsum", bufs=1)
                for i in range(24):
                    nc.tensor.matmul(wpsum[:], wmup[:, 0:128], wmup[:], start=True, stop=True)
                zsb = inp.tile([P, ZB, Ddim], F32)
                nc.sync.dma_start(out=zsb[:], in_=z_ext[:].rearrange("(nb p) i -> p nb i", p=P))
                xsb = inp.tile([P, MB, Ddim], F32)
                nc.sync.dma_start(out=xsb[:], in_=x_ext[:].rearrange("(mb p) i -> p mb i", p=P))
                wk = wp.tile([P, IO, Ddim], BF16)
                wv = wp.tile([P, IO, Ddim], BF16)
                wq = wp.tile([P, IO, Ddim], BF16)
                for io in range(IO):
                    ws = wst.tile([P, Ddim], F32, tag="ws", name=f"ws_k_{io}")
                    nc.scalar.dma_start(out=ws[:], in_=wk_ext[io * P:(io + 1) * P, :])
                    nc.vector.tensor_copy(wk[:, io, :], ws[:])
                zT = zp.tile([P, IO, ROWS], BF16)
                for io in range(IO):
                    for nb in range(ZB):
                        tp = tpp.tile([P, P], F32, tag="tp", name=f"tp_{nb}_{io}")
                        nc.tensor.transpose(tp[:], zsb[:, nb, io * P:(io + 1) * P], ident[:])
                        nc.vector.tensor_copy(zT[:, io, nb * P:(nb + 1) * P], tp[:])

                KTs = persist.tile([P, AO, ROWS], BF16)
                for ao in range(AO):
                    kp = pp.tile([P, ROWS], F32, tag="kp", name=f"kp_{ao}")
                    for io in range(IO):
                        nc.tensor.matmul(kp[:], wk[:, io, ao * P:(ao + 1) * P], zT[:, io, :],
                                         start=(io == 0), stop=(io == IO - 1))
                    nc.vector.tensor_scalar(KTs[:, ao, :], kp[:], bks[:, ao:ao + 1], None, OP.add)
                NH = SW // 2
                for nh in range(2):
                    nc.sync.dma_start(out=kt_bns[nh][:].rearrange("p (ao n) -> p ao n", n=NH),
                                      in_=KTs[:, :, nh * NH:(nh + 1) * NH])
                    nc.gpsimd.collective_compute(
                        "AllGather", OP.bypass, replica_groups=[list(range(NCORES))],
                        ins=[kt_bns[nh][:].opt()], outs=[kt_gns[nh][:].opt()])

                # wv/wq staged after K so their casts stay off the K critical path
                for wi, (eng, wtile, wext) in enumerate((
                        (nc.scalar, wv, wv_ext), (nc.gpsimd, wq, wq_ext))):
                    for io in range(IO):
                        ws = wst.tile([P, Ddim], F32, tag="ws", name=f"ws_{wi}_{io}")
                        eng.dma_start(out=ws[:], in_=wext[io * P:(io + 1) * P, :])
                        nc.vector.tensor_copy(wtile[:, io, :], ws[:])

                # Q^T projection (overlaps the K AllGather)
                xT = zp.tile([P, IO, ROWS], BF16)
                for io in range(IO):
                    for mb in range(MB):
                        tq = tpp.tile([P, P], F32, tag="tp", name=f"tq_{mb}_{io}")
                        nc.tensor.transpose(tq[:], xsb[:, mb, io * P:(io + 1) * P], ident[:])
                        nc.vector.tensor_copy(xT[:, io, mb * P:(mb + 1) * P], tq[:])
                for ao in range(AO):
                    qp = pp.tile([P, ROWS], F32, tag="kp", name=f"qp_{ao}")
                    for io in range(IO):
                        nc.tensor.matmul(qp[:], wq[:, io, ao * P:(ao + 1) * P], xT[:, io, :],
                                         start=(io == 0), stop=(io == IO - 1))
                    # fold the softmax 1/sqrt(D) into Q^T
                    nc.vector.tensor_scalar(QT[:, ao, :], qp[:], bqs[:, ao:ao + 1], float(scale),
                                            OP.add, OP.mult)

                Vs = persist.tile([P, VH, ZB, 512], BF16)
                for nb in range(ZB):
                    vp = pp.tile([P, Ddim], F32, tag="vp", name=f"vp_{nb}", bufs=1)
                    for io in range(IO):
                        for vh in range(VH):
                            nc.tensor.matmul(vp[:, vh * 512:(vh + 1) * 512],
                                             zT[:, io, nb * P:(nb + 1) * P],
                                             wv[:, io, vh * 512:(vh + 1) * 512],
                                             start=(io == 0), stop=(io == IO - 1))
                    for vh in range(VH):
                        nc.vector.tensor_tensor(Vs[:, vh, nb, :], vp[:, vh * 512:(vh + 1) * 512],
                                                bvb[:, vh * 512:(vh + 1) * 512], OP.add)
                for vh in range(VH):
                    nc.sync.dma_start(out=v_bds[vh][:], in_=Vs[:, vh])
                    nc.gpsimd.collective_compute(
                        "AllGather", OP.bypass, replica_groups=[list(range(NCORES))],
                        ins=[v_bds[vh][:].opt()], outs=[v_gds[vh][:].opt()])

            # ---------------- Phase 3: attention ----------------
            acc = persist.tile([P, MB, Ddim], F32)       # PV accumulator (SBUF)
            with tc.tile_pool(name="ktp", bufs=2) as ktp, \
                 tc.tile_pool(name="vtp", bufs=3) as vtp, \
                 tc.tile_pool(name="esp", bufs=8) as esp, \
                 tc.tile_pool(name="recp", bufs=1) as recp:
                # nq[p, t] = 1.0 where tile t is NOT this core's own shard
                nq = constp.tile([P, NT], F32)
                nc.vector.tensor_scalar(nq[:], r0t[:], 0.0, None, OP.not_equal)
                # precompute all masks up front (hides under the AllGather):
                # mk_all[t] keeps where (m-p) + (row0 - SW*t - 128j) <= 0, t != own
                mk_all = persist.tile([P, NT, JT * ROWS], BF16)
                mk_loc = persist.tile([P, JT * ROWS], BF16)
                with tc.tile_pool(name="iop", bufs=1) as iop:
                    iota1 = iop.tile([P, ROWS], F32)
                    nc.gpsimd.iota(iota1[:], pattern=[[1, ROWS]], base=0,
                                   channel_multiplier=-1,
                                   allow_small_or_imprecise_dtypes=True)
                    njt = iop.tile([P, NT * JT], F32)
                    nc.sync.dma_start(out=njt[:], in_=njt_d.ap())
                    r0tj = iop.tile([P, NT * JT], F32)
                    nc.vector.tensor_scalar(r0tj[:], njt[:], row0b[:], None, OP.add)
                    for j in range(JT):
                        nc.vector.tensor_scalar(mk_loc[:, j * ROWS:(j + 1) * ROWS], iota1[:],
                                                float(-128 * j), 0.0, OP.add, OP.is_le)
                    for t in range(NT):
                        for j in range(JT):
                            tj = t * JT + j
                            nc.vector.tensor_scalar(mk_all[:, t, j * ROWS:(j + 1) * ROWS],
                                                    iota1[:], r0tj[:, tj:tj + 1], 0.0,
                                                    OP.add, OP.is_le)
                        nc.vector.tensor_scalar(mk_all[:, t, :], mk_all[:, t, :],
                                                nq[:, t:t + 1], None, OP.mult)

                es_list = []
                recs = []
                # window 1: S passes + row-sums (psum: sp 2 + lt 4 + local-pv 2 = 8)
                with tc.tile_pool(name="spp", bufs=2, space="PSUM") as spp, \
                     tc.tile_pool(name="lpp", bufs=1, space="PSUM") as lpp, \
                     tc.tile_pool(name="plp", bufs=1, space="PSUM") as plp:
                    # one row-sum psum per 128-row m-chunk, bank-separated,
                    # accumulated across the local pass and all gathered tiles
                    lts = [lpp.tile([P, 8], F32, tag=f"lt{g}", name=f"lt{g}")
                           for g in range(MB)]

                    def attn_s(tag, kt_src, mk_ap, first, last, es_tag="es"):
                        es = esp.tile([P, JT, ROWS], BF16, tag=es_tag, name=f"es_{tag}")
                        for j in range(JT):
                            sp = spp.tile([P, ROWS], F32, tag="sp", name=f"sp_{tag}_{j}")
                            for io in range(IO):
                                nc.tensor.matmul(sp[:], kt_src[:, io, j * P:(j + 1) * P],
                                                 QT[:, io, :], start=(io == 0),
                                                 stop=(io == IO - 1))
                            nc.scalar.activation(es[:, j, :], sp[:], AF.Exp)
                        nc.vector.tensor_tensor(es[:].rearrange("p j m -> p (j m)"),
                                                es[:].rearrange("p j m -> p (j m)"),
                                                mk_ap, OP.mult)
                        for j in range(JT):
                            for g in range(MB):
                                nc.tensor.matmul(lts[g][:], es[:, j, g * P:(g + 1) * P],
                                                 ones8[:], start=(first and j == 0),
                                                 stop=(last and j == JT - 1))
                        return es

                    # local pre-pass on this core's own shard - overlaps the CCs
                    es_l = attn_s("loc", KTs, mk_loc[:], first=True, last=False,
                                  es_tag="esl")
                    for vh in range(VH):
                        pls = [plp.tile([P, 512], F32, tag=f"pl{mc}",
                                        name=f"pl_{vh}_{h}_{mc}")
                               for h in range(HLF) for mc in range(2)]
                        # 4 groups would need 4 banks; keep 2 live via h-order
                        for h in range(HLF):
                            for mc in range(2):
                                pv = pls[2 * h + mc]
                                for j in range(JT):
                                    nc.tensor.matmul(pv[:], es_l[:, j, h * 256 + mc * P:
                                                                  h * 256 + mc * P + P],
                                                     Vs[:, vh, j, :],
                                                     start=(j == 0), stop=(j == JT - 1))
                                nc.vector.tensor_copy(
                                    acc[:, 2 * h + mc, vh * 512:(vh + 1) * 512], pv[:])

                    # S pass, key-column halves: the first half's subtiles run
                    # while the second K half-gather is still in flight
                    JH = JT // 2
                    NH = SW // 2
                    for t in range(NT):
                        es = esp.tile([P, JT, ROWS], BF16, tag="es", name=f"es_g{t}")
                        es_list.append(es)
                    for nh in range(2):
                        for t in range(NT):
                            ktn = ktp.tile([P, IO, NH], BF16, tag=f"ktn{nh}",
                                           name=f"ktn{nh}_{t}")
                            nc.sync.dma_start(out=ktn[:], in_=kt_gns[nh][t])
                            es = es_list[t]
                            for jj in range(JH):
                                j = nh * JH + jj
                                sp = spp.tile([P, ROWS], F32, tag="sp", name=f"sp_{nh}_{t}_{j}")
                                for io in range(IO):
                                    nc.tensor.matmul(sp[:], ktn[:, io, jj * P:(jj + 1) * P],
                                                     QT[:, io, :], start=(io == 0),
                                                     stop=(io == IO - 1))
                                nc.scalar.activation(es[:, j, :], sp[:], AF.Exp)
                            efl = es[:].rearrange("p j m -> p (j m)")
                            nc.vector.tensor_tensor(
                                efl[:, nh * JH * ROWS:(nh + 1) * JH * ROWS],
                                efl[:, nh * JH * ROWS:(nh + 1) * JH * ROWS],
                                mk_all[:, t, nh * JH * ROWS:(nh + 1) * JH * ROWS], OP.mult)
                            for jj in range(JH):
                                j = nh * JH + jj
                                for g in range(MB):
                                    nc.tensor.matmul(lts[g][:], es[:, j, g * P:(g + 1) * P],
                                                     ones8[:], start=False,
                                                     stop=(nh == 1 and t == NT - 1
                                                           and jj == JH - 1))

                    # row-sum reciprocals straight from psum
                    for g in range(MB):
                        rec = recp.tile([P, 1], F32, tag=f"rec{g}", name=f"rec_{g}")
                        nc.vector.reciprocal(rec[:], lts[g][:, 0:1])
                        recs.append(rec)

                # window 2: PV over gathered tiles, one uninterrupted psum
                # accumulation per (m-chunk, value-half) - no mid-pass evictions
                with tc.tile_pool(name="pvg", bufs=2, space="PSUM") as pvg:
                    for vh in range(VH):
                        pvf = [pvg.tile([P, 512], F32, tag=f"pvf{g}",
                                        name=f"pvf_{vh}_{g}") for g in range(MB)]
                        for t in range(NT):
                            vtt = vtp.tile([P, JT, 512], BF16, tag="vtt",
                                           name=f"vtt_{vh}_{t}")
                            nc.sync.dma_start(out=vtt[:], in_=v_gds[vh][t])
                            for j in range(JT):
                                for g in range(MB):
                                    nc.tensor.matmul(pvf[g][:],
                                                     es_list[t][:, j, g * P:(g + 1) * P],
                                                     vtt[:, j, :],
                                                     start=(t == 0 and j == 0),
                                                     stop=(t == NT - 1 and j == JT - 1))
                        for g in range(MB):
                            vsl = slice(vh * 512, (vh + 1) * 512)
                            nc.vector.tensor_tensor(acc[:, g, vsl], acc[:, g, vsl],
                                                    pvf[g][:], OP.add)

                # normalize and write out
                for gmc in range(MB):
                    nc.vector.tensor_scalar(acc[:, gmc, :], acc[:, gmc, :], recs[gmc][:],
                                            None, OP.mult)
                nc.sync.dma_start(out=out_ext[:].rearrange("(mb p) v -> p mb v", p=P),
                                  in_=acc[:])
    nc.compile()
    return nc


_GRAPH_CACHE = {}


def _get_graph(Ldim=L, Ddim=D):
    key = (Ldim, Ddim)
    if key not in _GRAPH_CACHE:
        _GRAPH_CACHE[key] = build_graph(Ldim, Ddim)
    return _GRAPH_CACHE[key]


def kernel(x, z, Wq, bq, Wk, bk, Wv, bv):
    x = np.ascontiguousarray(np.asarray(x, dtype=np.float32))
    z = np.ascontiguousarray(np.asarray(z, dtype=np.float32))
    Ldim, Ddim = x.shape
    nc = _get_graph(Ldim, Ddim)
    ROWS = Ldim // NCORES
    common = {
        "Wq": np.ascontiguousarray(np.asarray(Wq, np.float32)),
        "bq": np.ascontiguousarray(np.asarray(bq, np.float32)),
        "Wk": np.ascontiguousarray(np.asarray(Wk, np.float32)),
        "bk": np.ascontiguousarray(np.asarray(bk, np.float32)),
        "Wv": np.ascontiguousarray(np.asarray(Wv, np.float32)),
        "bv": np.ascontiguousarray(np.asarray(bv, np.float32)),
    }
    in_maps = []
    for c in range(NCORES):
        m = dict(common)
        m["x"] = x[ROWS * c:ROWS * (c + 1)]
        m["z"] = z[ROWS * c:ROWS * (c + 1)]
        m["row0"] = np.array([ROWS * c], dtype=np.float32)
        in_maps.append(m)
    try:
        res = run_bass_kernel_spmd(nc, in_maps, core_ids=list(range(NCORES)))
    except Exception:
        # transient NRT device hiccups have been observed; one retry
        res = run_bass_kernel_spmd(nc, in_maps, core_ids=list(range(NCORES)))
    out = np.empty((Ldim, Ddim), dtype=np.float32)
    for c in range(NCORES):
        out[ROWS * c:ROWS * (c + 1)] = res.results[c]["out"]
    return out


# revision 34
# speedup vs baseline: 1.0382x; 1.0382x over previous
"""Distributed Trainium2 Bass kernel: masked (upper-triangular) attention.

reference (L=4096, D=1024, fp32):
    Q = x @ Wq + bq ; K = z @ Wk + bk ; V = z @ Wv + bv
    S = Q @ K.T ; S[row > col] = -inf
    out = softmax(S / sqrt(D)) @ V

Strategy (8 NeuronCores, one TRN2 chip, SPMD):
  - Sequence parallel on query rows: core c owns rows [512c, 512c+512).
  - K/V projection sharded over z rows (512/core), AllGathered in bf16
    (K stored transposed [D, L] blocked by shard, V natural [L, D]).
  - Attention computed as S^T tiles (keys on partitions) so the P^T needed by
    the PV matmul comes straight out of the softmax with no transposes.
  - Softmax without max-subtraction (scores here are O(1), exp can't overflow
    in fp32); mask applied multiplicatively after exp, built at runtime from
    an iota constant + a per-core row0 scalar input, keeping one graph valid
    for all cores (SPMD - no per-core control flow).
  - Matmuls in bf16 with fp32 PSUM accumulation (end-to-end rel err ~3e-3).
"""

import math

import numpy as np

import concourse.mybir as mybir
import concourse.tile as tile
from concourse import bacc
from concourse.bass_utils import run_bass_kernel_spmd

F32 = mybir.dt.float32
BF16 = mybir.dt.bfloat16
AF = mybir.ActivationFunctionType
OP = mybir.AluOpType
P = 128
NCORES = 8

L = 4096
D = 1024


def build_graph(Ldim=L, Ddim=D):
    nc = bacc.Bacc("TRN2", target_bir_lowering=False, debug=False, num_devices=NCORES)
    ROWS = Ldim // NCORES        # query rows per core
    MB = ROWS // P               # 128-row m-chunks per core (4)
    ZB = ROWS // P               # z-shard 128-row blocks (4)
    SW = ROWS                    # key-tile width == z-shard width (512)
    JT = SW // P                 # 128-row subtiles per key tile (4)
    NT = NCORES                  # one key tile per shard
    IO = Ddim // P               # contraction chunks (8)
    AO = Ddim // P               # d_attn 128-blocks (8)
    VH = Ddim // 512             # 512-wide value column halves (2)
    HLF = ROWS // 256            # 256-row halves for PV psum pressure (2)
    scale = 1.0 / math.sqrt(Ddim)

    x_ext = nc.declare_dram_parameter("x", [ROWS, Ddim], F32, isOutput=False)
    z_ext = nc.declare_dram_parameter("z", [ROWS, Ddim], F32, isOutput=False)
    wq_ext = nc.declare_dram_parameter("Wq", [Ddim, Ddim], F32, isOutput=False)
    wk_ext = nc.declare_dram_parameter("Wk", [Ddim, Ddim], F32, isOutput=False)
    wv_ext = nc.declare_dram_parameter("Wv", [Ddim, Ddim], F32, isOutput=False)
    bq_ext = nc.declare_dram_parameter("bq", [Ddim], F32, isOutput=False)
    bk_ext = nc.declare_dram_parameter("bk", [Ddim], F32, isOutput=False)
    bv_ext = nc.declare_dram_parameter("bv", [Ddim], F32, isOutput=False)
    row0_ext = nc.declare_dram_parameter("row0", [1], F32, isOutput=False)
    out_ext = nc.declare_dram_parameter("out", [ROWS, Ddim], F32, isOutput=True)

    ident_d = nc.inline_tensor(np.eye(P, dtype=np.float32), name="ident_c")
    ones_d = nc.inline_tensor(np.ones((P, 8), np.float32), name="ones_c")
    # mask keeps where (m - p) + (row0 - SW*t - 128j) <= 0
    njt_np = np.broadcast_to(
        -(float(SW) * np.arange(NT)[:, None] + 128.0 * np.arange(JT)[None, :])
        .astype(np.float32).reshape(1, NT * JT), (P, NT * JT)).copy()
    njt_d = nc.inline_tensor(njt_np, name="njt_c")
    nSWt_d = nc.inline_tensor(
        np.broadcast_to((-float(SW) * np.arange(NT, dtype=np.float32))[None, :], (P, NT)).copy(),
        name="nswt_c")

    with tile.TileContext(nc) as tc:
        with tc.tile_pool(name="const", bufs=1) as constp, \
             tc.tile_pool(name="persist", bufs=1) as persist, \
             tc.tile_pool(name="dram", bufs=1, space="DRAM") as dram:
            ident = constp.tile([P, P], F32)
            nc.sync.dma_start(out=ident[:], in_=ident_d.ap())
            ones_f = constp.tile([P, 8], F32)
            nc.sync.dma_start(out=ones_f[:], in_=ones_d.ap())
            ones8 = constp.tile([P, 8], BF16)
            nc.vector.tensor_copy(ones8[:], ones_f[:])
            bvb = constp.tile([P, Ddim], F32)
            nc.sync.dma_start(out=bvb[:], in_=bv_ext[:].partition_broadcast(P))
            bqs = constp.tile([P, AO], F32)
            nc.sync.dma_start(out=bqs[:], in_=bq_ext[:].rearrange("(ao p) -> p ao", p=P))
            bks = constp.tile([P, AO], F32)
            nc.sync.dma_start(out=bks[:], in_=bk_ext[:].rearrange("(ao p) -> p ao", p=P))
            row0b = constp.tile([P, 1], F32)
            nc.sync.dma_start(out=row0b[:], in_=row0_ext[:].partition_broadcast(P))
            nswt = constp.tile([P, NT], F32)
            nc.sync.dma_start(out=nswt[:], in_=nSWt_d.ap())
            r0t = constp.tile([P, NT], F32)
            nc.vector.tensor_scalar(r0t[:], nswt[:], row0b[:], None, OP.add)

            QT = persist.tile([P, IO, ROWS], BF16)
            KW = AO * ROWS               # flat K width per partition
            VW = ZB * Ddim               # flat V width per partition
            kt_bd = dram.tile([P, KW], BF16)
            v_bds = [dram.tile([P, VW // VH], BF16, name=f"v_bd{vh}") for vh in range(VH)]
            kt_gd = dram.tile([NCORES, P, KW], BF16)
            v_gds = [dram.tile([NCORES, P, VW // VH], BF16, name=f"v_gd{vh}") for vh in range(VH)]

            # ------- Phase 1+2: projections of own shards; K/V AllGathered -------
            with tc.tile_pool(name="inp", bufs=1) as inp, \
                 tc.tile_pool(name="wst", bufs=3) as wst, \
                 tc.tile_pool(name="wkv", bufs=1) as wp, \
                 tc.tile_pool(name="zp", bufs=1) as zp, \
                 tc.tile_pool(name="tpp", bufs=2, space="PSUM") as tpp, \
                 tc.tile_pool(name="pp", bufs=2, space="PSUM") as pp:
                wmup = wst.tile([P, 512], BF16, tag="wm", name="wmup")
                nc.vector.memset(wmup[:], 0.0)
                wpsum = tpp.tile([P, 512], F32, tag="wm", name="wpsum", bufs=1)
                for i in range(24):
                    nc.tensor.matmul(wpsum[:], wmup[:, 0:128], wmup[:], start=True, stop=True)
                zsb = inp.tile([P, ZB, Ddim], F32)
                nc.sync.dma_start(out=zsb[:], in_=z_ext[:].rearrange("(nb p) i -> p nb i", p=P))
                xsb = inp.tile([P, MB, Ddim], F32)
                nc.sync.dma_start(out=xsb[:], in_=x_ext[:].rearrange("(mb p) i -> p mb i", p=P))
                wk = wp.tile([P, IO, Ddim], BF16)
                wv = wp.tile([P, IO, Ddim], BF16)
                wq = wp.tile([P, IO, Ddim], BF16)
                for io in range(IO):
                    ws = wst.tile([P, Ddim], F32, tag="ws", name=f"ws_k_{io}")
                    nc.scalar.dma_start(out=ws[:], in_=wk_ext[io * P:(io + 1) * P, :])
                    nc.vector.tensor_copy(wk[:, io, :], ws[:])
                zT = zp.tile([P, IO, ROWS], BF16)
                for io in range(IO):
                    for nb in range(ZB):
                        tp = tpp.tile([P, P], F32, tag="tp", name=f"tp_{nb}_{io}")
                        nc.tensor.transpose(tp[:], zsb[:, nb, io * P:(io + 1) * P], ident[:])
                        nc.vector.tensor_copy(zT[:, io, nb * P:(nb + 1) * P], tp[:])

                KTs = persist.tile([P, AO, ROWS], BF16)
                for ao in range(AO):
                    kp = pp.tile([P, ROWS], F32, tag="kp", name=f"kp_{ao}")
                    for io in range(IO):
                        nc.tensor.matmul(kp[:], wk[:, io, ao * P:(ao + 1) * P], zT[:, io, :],
                                         start=(io == 0), stop=(io == IO - 1))
                    nc.vector.tensor_scalar(KTs[:, ao, :], kp[:], bks[:, ao:ao + 1], None, OP.add)
                nc.sync.dma_start(out=kt_bd[:], in_=KTs[:])
                nc.gpsimd.collective_compute(
                    "AllGather", OP.bypass, replica_groups=[list(range(NCORES))],
                    ins=[kt_bd[:].opt()], outs=[kt_gd[:].opt()])

                # wv/wq staged after K so their casts stay off the K critical path
                for wi, (eng, wtile, wext) in enumerate((
                        (nc.scalar, wv, wv_ext), (nc.gpsimd, wq, wq_ext))):
                    for io in range(IO):
                        ws = wst.tile([P, Ddim], F32, tag="ws", name=f"ws_{wi}_{io}")
                        eng.dma_start(out=ws[:], in_=wext[io * P:(io + 1) * P, :])
                        nc.vector.tensor_copy(wtile[:, io, :], ws[:])

                # Q^T projection (overlaps the K AllGather)
                xT = zp.tile([P, IO, ROWS], BF16)
                for io in range(IO):
                    for mb in range(MB):
                        tq = tpp.tile([P, P], F32, tag="tp", name=f"tq_{mb}_{io}")
                        nc.tensor.transpose(tq[:], xsb[:, mb, io * P:(io + 1) * P], ident[:])
                        nc.vector.tensor_copy(xT[:, io, mb * P:(mb + 1) * P], tq[:])
                for ao in range(AO):
                    qp = pp.tile([P, ROWS], F32, tag="kp", name=f"qp_{ao}")
                    for io in range(IO):
                        nc.tensor.matmul(qp[:], wq[:, io, ao * P:(ao + 1) * P], xT[:, io, :],
                                         start=(io == 0), stop=(io == IO - 1))
                    # fold the softmax 1/sqrt(D) into Q^T
                    nc.vector.tensor_scalar(QT[:, ao, :], qp[:], bqs[:, ao:ao + 1], float(scale),
                                            OP.add, OP.mult)

                Vs = persist.tile([P, VH, ZB, 512], BF16)
                for nb in range(ZB):
                    vp = pp.tile([P, Ddim], F32, tag="vp", name=f"vp_{nb}", bufs=1)
                    for io in range(IO):
                        for vh in range(VH):
                            nc.tensor.matmul(vp[:, vh * 512:(vh + 1) * 512],
                                             zT[:, io, nb * P:(nb + 1) * P],
                                             wv[:, io, vh * 512:(vh + 1) * 512],
                                             start=(io == 0), stop=(io == IO - 1))
                    for vh in range(VH):
                        nc.vector.tensor_tensor(Vs[:, vh, nb, :], vp[:, vh * 512:(vh + 1) * 512],
                                                bvb[:, vh * 512:(vh + 1) * 512], OP.add)
                for vh in range(VH):
                    nc.sync.dma_start(out=v_bds[vh][:], in_=Vs[:, vh])
                    nc.gpsimd.collective_compute(
                        "AllGather", OP.bypass, replica_groups=[list(range(NCORES))],
                        ins=[v_bds[vh][:].opt()], outs=[v_gds[vh][:].opt()])

            # ---------------- Phase 3: attention ----------------
            acc = persist.tile([P, MB, Ddim], F32)       # PV accumulator (SBUF)
            with tc.tile_pool(name="ktp", bufs=2) as ktp, \
                 tc.tile_pool(name="vtp", bufs=3) as vtp, \
                 tc.tile_pool(name="esp", bufs=8) as esp, \
                 tc.tile_pool(name="recp", bufs=1) as recp:
                # nq[p, t] = 1.0 where tile t is NOT this core's own shard
                nq = constp.tile([P, NT], F32)
                nc.vector.tensor_scalar(nq[:], r0t[:], 0.0, None, OP.not_equal)
                # precompute all masks up front (hides under the AllGather):
                # mk_all[t] keeps where (m-p) + (row0 - SW*t - 128j) <= 0, t != own
                mk_all = persist.tile([P, NT, JT * ROWS], BF16)
                mk_loc = persist.tile([P, JT * ROWS], BF16)
                with tc.tile_pool(name="iop", bufs=1) as iop:
                    iota1 = iop.tile([P, ROWS], F32)
                    nc.gpsimd.iota(iota1[:], pattern=[[1, ROWS]], base=0,
                                   channel_multiplier=-1,
                                   allow_small_or_imprecise_dtypes=True)
                    njt = iop.tile([P, NT * JT], F32)
                    nc.sync.dma_start(out=njt[:], in_=njt_d.ap())
                    r0tj = iop.tile([P, NT * JT], F32)
                    nc.vector.tensor_scalar(r0tj[:], njt[:], row0b[:], None, OP.add)
                    for j in range(JT):
                        nc.vector.tensor_scalar(mk_loc[:, j * ROWS:(j + 1) * ROWS], iota1[:],
                                                float(-128 * j), 0.0, OP.add, OP.is_le)
                    for t in range(NT):
                        for j in range(JT):
                            tj = t * JT + j
                            nc.vector.tensor_scalar(mk_all[:, t, j * ROWS:(j + 1) * ROWS],
                                                    iota1[:], r0tj[:, tj:tj + 1], 0.0,
                                                    OP.add, OP.is_le)
                        nc.vector.tensor_scalar(mk_all[:, t, :], mk_all[:, t, :],
                                                nq[:, t:t + 1], None, OP.mult)

                es_list = []
                recs = []
                # window 1: S passes + row-sums (psum: sp 2 + lt 4 + local-pv 2 = 8)
                with tc.tile_pool(name="spp", bufs=2, space="PSUM") as spp, \
                     tc.tile_pool(name="lpp", bufs=1, space="PSUM") as lpp, \
                     tc.tile_pool(name="plp", bufs=1, space="PSUM") as plp:
                    # one row-sum psum per 128-row m-chunk, bank-separated,
                    # accumulated across the local pass and all gathered tiles
                    lts = [lpp.tile([P, 8], F32, tag=f"lt{g}", name=f"lt{g}")
                           for g in range(MB)]

                    def attn_s(tag, kt_src, mk_ap, first, last, es_tag="es"):
                        es = esp.tile([P, JT, ROWS], BF16, tag=es_tag, name=f"es_{tag}")
                        for j in range(JT):
                            sp = spp.tile([P, ROWS], F32, tag="sp", name=f"sp_{tag}_{j}")
                            for io in range(IO):
                                nc.tensor.matmul(sp[:], kt_src[:, io, j * P:(j + 1) * P],
                                                 QT[:, io, :], start=(io == 0),
                                                 stop=(io == IO - 1))
                            nc.scalar.activation(es[:, j, :], sp[:], AF.Exp)
                        nc.vector.tensor_tensor(es[:].rearrange("p j m -> p (j m)"),
                                                es[:].rearrange("p j m -> p (j m)"),
                                                mk_ap, OP.mult)
                        for j in range(JT):
                            for g in range(MB):
                                nc.tensor.matmul(lts[g][:], es[:, j, g * P:(g + 1) * P],
                                                 ones8[:], start=(first and j == 0),
                                                 stop=(last and j == JT - 1))
                        return es

                    # local pre-pass on this core's own shard - overlaps the CCs
                    es_l = attn_s("loc", KTs, mk_loc[:], first=True, last=False,
                                  es_tag="esl")
                    for vh in range(VH):
                        pls = [plp.tile([P, 512], F32, tag=f"pl{mc}",
                                        name=f"pl_{vh}_{h}_{mc}")
                               for h in range(HLF) for mc in range(2)]
                        # 4 groups would need 4 banks; keep 2 live via h-order
                        for h in range(HLF):
                            for mc in range(2):
                                pv = pls[2 * h + mc]
                                for j in range(JT):
                                    nc.tensor.matmul(pv[:], es_l[:, j, h * 256 + mc * P:
                                                                  h * 256 + mc * P + P],
                                                     Vs[:, vh, j, :],
                                                     start=(j == 0), stop=(j == JT - 1))
                                nc.vector.tensor_copy(
                                    acc[:, 2 * h + mc, vh * 512:(vh + 1) * 512], pv[:])

                    # S pass for all gathered key tiles (overlaps the V AllGathers)
                    for t in range(NT):
                        ktt = ktp.tile([P, IO, SW], BF16, tag="ktt", name=f"ktt_{t}")
                        nc.sync.dma_start(out=ktt[:], in_=kt_gd[t])
                        es_list.append(attn_s(f"g{t}", ktt, mk_all[:, t, :],
                                              first=False, last=(t == NT - 1)))

                    # row-sum reciprocals straight from psum
                    for g in range(MB):
                        rec = recp.tile([P, 1], F32, tag=f"rec{g}", name=f"rec_{g}")
                        nc.vector.reciprocal(rec[:], lts[g][:, 0:1])
                        recs.append(rec)

                # window 2: PV over gathered tiles, one uninterrupted psum
                # accumulation per (m-chunk, value-half) - no mid-pass evictions
                with tc.tile_pool(name="pvg", bufs=2, space="PSUM") as pvg:
                    for vh in range(VH):
                        pvf = [pvg.tile([P, 512], F32, tag=f"pvf{g}",
                                        name=f"pvf_{vh}_{g}") for g in range(MB)]
                        for t in range(NT):
                            vtt = vtp.tile([P, JT, 512], BF16, tag="vtt",
                                           name=f"vtt_{vh}_{t}")
                            nc.sync.dma_start(out=vtt[:], in_=v_gds[vh][t])
                            for j in range(JT):
                                for g in range(MB):
                                    nc.tensor.matmul(pvf[g][:],
                                                     es_list[t][:, j, g * P:(g + 1) * P],
                                                     vtt[:, j, :],
                                                     start=(t == 0 and j == 0),
                                                     stop=(t == NT - 1 and j == JT - 1))
                        for g in range(MB):
                            vsl = slice(vh * 512, (vh + 1) * 512)
                            nc.vector.tensor_tensor(acc[:, g, vsl], acc[:, g, vsl],
                                                    pvf[g][:], OP.add)

                # normalize and write out
                for gmc in range(MB):
                    nc.vector.tensor_scalar(acc[:, gmc, :], acc[:, gmc, :], recs[gmc][:],
                                            None, OP.mult)
                nc.sync.dma_start(out=out_ext[:].rearrange("(mb p) v -> p mb v", p=P),
                                  in_=acc[:])
    nc.compile()
    return nc


_GRAPH_CACHE = {}


def _get_graph(Ldim=L, Ddim=D):
    key = (Ldim, Ddim)
    if key not in _GRAPH_CACHE:
        _GRAPH_CACHE[key] = build_graph(Ldim, Ddim)
    return _GRAPH_CACHE[key]


def kernel(x, z, Wq, bq, Wk, bk, Wv, bv):
    x = np.ascontiguousarray(np.asarray(x, dtype=np.float32))
    z = np.ascontiguousarray(np.asarray(z, dtype=np.float32))
    Ldim, Ddim = x.shape
    nc = _get_graph(Ldim, Ddim)
    ROWS = Ldim // NCORES
    common = {
        "Wq": np.ascontiguousarray(np.asarray(Wq, np.float32)),
        "bq": np.ascontiguousarray(np.asarray(bq, np.float32)),
        "Wk": np.ascontiguousarray(np.asarray(Wk, np.float32)),
        "bk": np.ascontiguousarray(np.asarray(bk, np.float32)),
        "Wv": np.ascontiguousarray(np.asarray(Wv, np.float32)),
        "bv": np.ascontiguousarray(np.asarray(bv, np.float32)),
    }
    in_maps = []
    for c in range(NCORES):
        m = dict(common)
        m["x"] = x[ROWS * c:ROWS * (c + 1)]
        m["z"] = z[ROWS * c:ROWS * (c + 1)]
        m["row0"] = np.array([ROWS * c], dtype=np.float32)
        in_maps.append(m)
    try:
        res = run_bass_kernel_spmd(nc, in_maps, core_ids=list(range(NCORES)))
    except Exception:
        # transient NRT device hiccups have been observed; one retry
        res = run_bass_kernel_spmd(nc, in_maps, core_ids=list(range(NCORES)))
    out = np.empty((Ldim, Ddim), dtype=np.float32)
    for c in range(NCORES):
        out[ROWS * c:ROWS * (c + 1)] = res.results[c]["out"]
    return out
